# revision 40
# baseline (speedup 1.0000x reference)
"""MultiHeadAttention Trainium2 kernel (8-core SPMD, collective-free).

Problem: B=4, T=2048, E=1024, H=16, D=64 multi-head self-attention
(torch-style Linear projections, softmax over keys, output projection).

Sharding: core c handles batch b=c//2 and head-half hh=c%2 (8 of the 16
heads) over ALL 2048 tokens.  Q/K/V projections and attention are
computed only for the core's own heads (no duplicated work).  The final
output projection is split over the CONTRACTION dim: each core multiplies
its own 512 attention features by its own 512 rows of Wo.T, producing a
full-width [T, E] PARTIAL output; the host sums the two partials of each
batch pair (out[b] = part[2b] + part[2b+1], bias folded into the even
core's partial).  No device collectives at all.

Device pipeline per core (storage bf16, all accumulation fp32), emitted
as an explicitly software-pipelined "weave" over 32 units
(8 local heads x 4 query blocks) -- engines execute their instruction
streams in order, so overlap must exist at emission time:
  - per unit: 8 kc-pair score groups (K=64 matmuls, scores^T layout),
    each followed by one exp ACT [128,1024] (scale=1/8 fused, no max
    subtraction -- logits are bounded), interleaved with the previous
    unit's PV matmuls and one filler chunk (q/k projection or output
    projection work) drawn from a queue,
  - PV lhsT = [v_h | 1]: psum row 64 accumulates the softmax denominator
    for free; K=1 float32r ones-matmul broadcasts it across partitions;
    DVE approx-reciprocal (~51 ULP) + multiply normalizes,
  - odd local heads' outputs are partition-shifted 0:64 -> 64:128 with a
    small SBUF->SBUF DMA so outT keeps a feature-major layout,
  - per query block (once all 8 local heads' PV are done): 4 token
    chunks of partial output projection (contraction = local features).
"""

import os
import sys
from contextlib import ExitStack

import numpy as np
import ml_dtypes

for _p in ("/opt/trn_rl_repo", "/root/.axon_site/_ro/trn_rl_repo"):
    if os.path.isdir(_p) and _p not in sys.path:
        sys.path.insert(0, _p)

import concourse.bass as bass  # noqa: E402,F401
from concourse import bacc  # noqa: E402
import concourse.tile as tile  # noqa: E402
from concourse import mybir  # noqa: E402
from concourse.bass_utils import run_bass_kernel_spmd  # noqa: E402

# ---- problem constants (hardcoded; kernel.py must be self-contained) ----
B, T, E, H, D = 4, 2048, 1024, 16, 64
P = 128
NCORES = 8
HL = H // 2          # 8 local heads per core
FE = HL * D          # 512 local features
EC = E // P          # 8 e-chunks (contraction chunks for projections)
FCH = FE // P        # 4 local feature chunks (head pairs)
KC = T // P          # 16 key-token chunks
QB = T // 512        # 4 query blocks
TC = T // P          # 16 output token chunks

BF = mybir.dt.bfloat16
F32 = mybir.dt.float32
F32R = mybir.dt.float32r
AF = mybir.ActivationFunctionType
ALU = mybir.AluOpType

SECTIONS = []        # (name, first_instruction_index) markers for profiling
USE_TILE_POS = os.environ.get("KERNEL_NO_TILEPOS", "0") != "1"
REPEAT = int(os.environ.get("KERNEL_REPEAT", "1"))


def build_program():
    nc = bacc.Bacc("TRN2", target_bir_lowering=False, debug=False,
                   num_devices=NCORES)

    def mark(name):
        SECTIONS.append((name, len(nc.inst_map)))

    xt_d = nc.dram_tensor("xt", [EC, P, T], BF, kind="ExternalInput").ap()
    wqt_d = nc.dram_tensor("wqt", [EC, P, FE], BF, kind="ExternalInput").ap()
    wkt_d = nc.dram_tensor("wkt", [EC, P, FE], BF, kind="ExternalInput").ap()
    wvt_d = nc.dram_tensor("wvt", [EC, P, FE], BF, kind="ExternalInput").ap()
    wot_d = nc.dram_tensor("wot", [FCH, P, E], BF, kind="ExternalInput").ap()
    bq_d = nc.dram_tensor("bq", [FCH, P], F32, kind="ExternalInput").ap()
    bk_d = nc.dram_tensor("bk", [FCH, P], F32, kind="ExternalInput").ap()
    bvb_d = nc.dram_tensor("bvb", [P, FE], F32, kind="ExternalInput").ap()
    bob_d = nc.dram_tensor("bob", [P, E], F32, kind="ExternalInput").ap()
    ones_d = nc.dram_tensor("ones", [P, 64], F32R, kind="ExternalInput").ap()
    out_d = nc.dram_tensor("out", [TC, P, E], F32,
                           kind="ExternalOutput").ap()

    with tile.TileContext(nc) as tc, ExitStack() as ctx:
        persist = ctx.enter_context(tc.tile_pool(name="persist", bufs=1))
        wq_pool = ctx.enter_context(tc.tile_pool(name="wq", bufs=2))
        wv_pool = ctx.enter_context(tc.tile_pool(name="wv", bufs=1))
        small = ctx.enter_context(tc.tile_pool(name="small", bufs=2))
        otmp_pool = ctx.enter_context(tc.tile_pool(name="otmp", bufs=1))
        fin_pool = ctx.enter_context(tc.tile_pool(name="finp", bufs=4))
        psc = ctx.enter_context(tc.tile_pool(name="psc", bufs=3, space="PSUM"))
        ppv = ctx.enter_context(tc.tile_pool(name="ppv", bufs=2, space="PSUM"))

        def sc_slot():
            return psc.tile([P, 2, 512], F32, tag="sc", name="sc")

        # ---------------- persistent SBUF tensors ----------------
        xt_sb = persist.tile([P, EC, T], BF, tag="xt")          # 32K
        bq_sb = persist.tile([P, FCH], F32, tag="bq")
        bk_sb = persist.tile([P, FCH], F32, tag="bk")
        bvb_sb = persist.tile([P, FE], F32, tag="bvb")          # 2K
        bob_sb = persist.tile([P, E], F32, tag="bob")           # 4K
        wot_sb = persist.tile([P, FCH, E], BF, tag="wot")       # 8K
        qt_sb = persist.tile([P, FCH, T], BF, tag="qt")         # 16K
        kt_sb = persist.tile([P, FCH, T], BF, tag="kt")         # 16K
        vaug = persist.tile([P, KC, HL * 65], BF, tag="vaug")   # 16.3K
        outT = persist.tile([P, FCH, T], BF, tag="outT")        # 16K
        ones_sb = persist.tile([P, 64], F32R, tag="ones")
        eh2 = [persist.tile([P, KC, 512], BF, tag="eh0", name="eh0"),
               persist.tile([P, KC, 512], BF, tag="eh1", name="eh1")]

        mark('in_dma')
        # ---------------- input DMAs ----------------
        # (xt is deferred until after the first weight DMAs in emit_body)
        nc.sync.dma_start(bq_sb[:], bq_d.rearrange("f p -> p f"))
        nc.sync.dma_start(bk_sb[:], bk_d.rearrange("f p -> p f"))
        nc.sync.dma_start(bvb_sb[:], bvb_d)
        nc.sync.dma_start(ones_sb[:], ones_d)
        vaug_h = vaug.rearrange("p k (h c) -> p k h c", c=65)
        for h in range(HL):
            nc.gpsimd.memset(vaug_h[:, :, h, 64:65], 1.0)

        # units: query-block major so each block's output projection can
        # start as early as possible; within a block the odd head of each
        # pair goes first so the last unit's normalize skips the
        # partition-shift DMA (shorter serial tail)
        HL_ORDER = [1, 0, 3, 2, 5, 4, 7, 6]
        UNITS = [(hl, qb) for qb in range(QB) for hl in HL_ORDER]

        filler = []

        def drain_filler(n=1):
            for _ in range(n):
                if not filler:
                    return
                filler.pop(0)()

        def queue_qk(fc, k_first=False):
            """qT/kT projection for local feature chunk fc as fillers.
            k_first orders [k:0-1024, q:0-1024] first so the first unit's
            scores can start after 2 of the 4 groups."""
            fs = slice(fc * P, (fc + 1) * P)
            box = {}

            def dma_w():
                box["wqf"] = wq_pool.tile([P, EC, P], BF, tag="wqf",
                                          name="wqf")
                box["wkf"] = wq_pool.tile([P, EC, P], BF, tag="wkf",
                                          name="wkf")
                for ec in range(EC):
                    nc.sync.dma_start(box["wqf"][:, ec, :], wqt_d[ec, :, fs])
                    nc.sync.dma_start(box["wkf"][:, ec, :], wkt_d[ec, :, fs])
            filler.append(dma_w)

            def group(kind, tp):
                w_key = "wqf" if kind == "q" else "wkf"
                dest = qt_sb if kind == "q" else kt_sb
                bias = bq_sb if kind == "q" else bk_sb
                ps_box = {}

                def mms(lo, hi):
                    def _f():
                        if "ps" not in ps_box:
                            ps_box["ps"] = sc_slot()
                        ps = ps_box["ps"]
                        for i in range(2):
                            tb = 2 * tp + i
                            for ec in range(lo, hi):
                                nc.tensor.matmul(
                                    ps[:, i, :], lhsT=box[w_key][:, ec, :],
                                    rhs=xt_sb[:, ec, tb * 512:(tb + 1) * 512],
                                    start=(ec == 0), stop=(ec == EC - 1),
                                )
                    return _f
                filler.append(mms(0, 4))
                filler.append(mms(4, 8))

                def evac():
                    nc.vector.tensor_scalar_add(
                        dest[:, fc, tp * 1024:(tp + 1) * 1024],
                        ps_box["ps"].rearrange("p a b -> p (a b)"),
                        bias[:, fc: fc + 1],
                    )
                filler.append(evac)
            order = ([("k", 0), ("q", 0), ("k", 1), ("q", 1)] if k_first
                     else [("q", 0), ("q", 1), ("k", 0), ("k", 1)])
            for kind, tp in order:
                group(kind, tp)

        def queue_qk0_granular():
            """Bootstrap variant of queue_qk(0): 512-token sub-groups so
            the first scores (kc 0-3) only wait on the first token block's
            k projection, and later k blocks stream in one group ahead of
            the score groups that read them."""
            fs = slice(0, P)
            box = {}

            def dma_w():
                box["wqf"] = wq_pool.tile([P, EC, P], BF, tag="wqf",
                                          name="wqf")
                box["wkf"] = wq_pool.tile([P, EC, P], BF, tag="wkf",
                                          name="wkf")
                for ec in range(EC):
                    nc.sync.dma_start(box["wqf"][:, ec, :], wqt_d[ec, :, fs])
                    nc.sync.dma_start(box["wkf"][:, ec, :], wkt_d[ec, :, fs])
            filler.append(dma_w)

            def sub(kind, tb):
                w_key = "wqf" if kind == "q" else "wkf"
                dest = qt_sb if kind == "q" else kt_sb
                bias = bq_sb if kind == "q" else bk_sb
                ps_box = {}

                def mms():
                    ps_box["ps"] = sc_slot()
                    for ec in range(EC):
                        nc.tensor.matmul(
                            ps_box["ps"][:, tb % 2, :],
                            lhsT=box[w_key][:, ec, :],
                            rhs=xt_sb[:, ec, tb * 512:(tb + 1) * 512],
                            start=(ec == 0), stop=(ec == EC - 1),
                        )
                filler.append(mms)

                def evac():
                    nc.vector.tensor_scalar_add(
                        dest[:, 0, tb * 512:(tb + 1) * 512],
                        ps_box["ps"][:, tb % 2, :], bias[:, 0:1])
                filler.append(evac)
            for kind, tb in [("k", 0), ("q", 0), ("k", 1), ("k", 2),
                             ("k", 3), ("q", 1), ("q", 2), ("q", 3)]:
                sub(kind, tb)

        def queue_outproj(qb):
            """partial final[t, ALL E cols] for the 4 token chunks of
            block qb (contraction = this core's 512 local features)."""
            for tcl in range(4):
                tc_ = qb * 4 + tcl
                ps_box = {}

                def mms(half, tc_=tc_, ps_box=ps_box):
                    def _f():
                        if "ps" not in ps_box:
                            ps_box["ps"] = sc_slot()
                        ps = ps_box["ps"]
                        for fc in range(FCH):
                            nc.tensor.matmul(
                                ps[:, half, :],
                                lhsT=outT[:, fc, tc_ * P:(tc_ + 1) * P],
                                rhs=wot_sb[:, fc, half * 512:(half + 1) * 512],
                                start=(fc == 0), stop=(fc == FCH - 1),
                            )
                    return _f
                filler.append(mms(0))
                filler.append(mms(1))

                def evac(tc_=tc_, ps_box=ps_box):
                    fin = fin_pool.tile([P, E], F32, tag="fin", name="fin")
                    nc.vector.tensor_tensor(
                        fin[:], ps_box["ps"].rearrange("p a b -> p (a b)"),
                        bob_sb[:], ALU.add)
                    # two half-width stores land on two DMA engines: the
                    # real store is ~23us/engine for 512KB, and the last
                    # one gates kernel end
                    nc.sync.dma_start(out_d[tc_][:, 0:512], fin[:, 0:512])
                    nc.sync.dma_start(out_d[tc_][:, 512:E], fin[:, 512:E])
                filler.append(evac)

        pv_state = {}

        def pv_mms(ui, kc):
            hl, qb = UNITS[ui]
            nc.tensor.matmul(
                pv_state[ui]["po"][0:65, :], lhsT=vaug_h[:, kc, hl, :],
                rhs=eh2[ui % 2][:, kc, :],
                start=(kc == 0), stop=(kc == KC - 1),
            )

        def pv_copy(ui):
            """Stage the denominator row to SBUF as soon as PV(ui) is done
            (the DVE copy then has ~2 score groups to land before the psR
            matmul needs it -- emitted together they stall PE ~650ns/unit)."""
            st = pv_state[ui]
            st["srb"] = small.tile([P, 512], F32R, tag="srb", name="srb")
            nc.vector.tensor_copy(st["srb"][64:65, :], st["po"][64:65, :])

        def pv_norm(ui):
            hl, qb = UNITS[ui]
            hp, par = hl // 2, hl % 2
            qs = slice(qb * 512, (qb + 1) * 512)
            st = pv_state.pop(ui)
            po, srb = st["po"], st["srb"]
            rec = small.tile([P, 512], F32, tag="rec", name="rec")
            psR = sc_slot()
            nc.tensor.matmul(psR[0:64, 0, :], lhsT=ones_sb[64:65, :],
                             rhs=srb[64:65, :], start=True, stop=True)
            nc.vector.reciprocal_approx_fast(rec[0:64, :], psR[0:64, 0, :])
            if par == 0:
                nc.vector.tensor_tensor(outT[0:64, hp, qs], po[0:64, :],
                                        rec[0:64, :], ALU.mult)
            else:
                ot = otmp_pool.tile([P, 512], BF, tag="ot", name="ot")
                nc.vector.tensor_tensor(ot[0:64, :], po[0:64, :],
                                        rec[0:64, :], ALU.mult)
                nc.sync.dma_start(outT[64:128, hp, qs], ot[0:64, :])

        def pv_finish(ui):
            pv_copy(ui)
            pv_norm(ui)

        def scores_group(ui, g):
            """Unit ui's kc-pair g: two K=64 score matmuls + one exp ACT."""
            hl, qb = UNITS[ui]
            hp, par = hl // 2, hl % 2
            qs = slice(qb * 512, (qb + 1) * 512)
            rows = slice(0, 64) if par == 0 else slice(64, 128)
            tp = (dict(tile_position=(0, 0)) if par == 0 else
                  dict(tile_position=(64, 0))) if USE_TILE_POS else {}
            eh = eh2[ui % 2]
            ps2 = sc_slot()
            for i in range(2):
                kc = 2 * g + i
                kslc = slice(kc * P, (kc + 1) * P)
                nc.tensor.matmul(
                    ps2[:, i, :], lhsT=kt_sb[rows, hp, kslc],
                    rhs=qt_sb[rows, hp, qs],
                    start=True, stop=True, **tp,
                )
            nc.scalar.activation(eh[:, 2 * g: 2 * g + 2, :], ps2[:],
                                 AF.Exp, scale=0.125)

        norm_pending = []

        def weave_unit(ui, do_scores=True):
            """Emit unit ui's scores+exp interleaved with unit ui-1's PV
            and filler chunks; the unit-before-last's deferred normalize
            lands at group 2 (its copy/psR dependencies are long done)."""
            if not do_scores:
                prev, ui = ui, None
            else:
                prev = ui - 1 if ui > 0 else None
            if prev is not None and prev not in pv_state:
                prev = None
            for g in range(KC // 2):
                if g == 2 and norm_pending:
                    u = norm_pending.pop(0)
                    pv_norm(u)
                    hlu, qbu = UNITS[u]
                    if hlu == HL_ORDER[-1] and qbu < QB - 1:
                        # block qbu's outT is now complete
                        queue_outproj(qbu)
                if ui is not None:
                    scores_group(ui, g)
                if prev is not None:
                    pv_mms(prev, 2 * g)
                    pv_mms(prev, 2 * g + 1)
                drain_filler(1)
            if prev is not None:
                pv_copy(prev)
                norm_pending.append(prev)
            if ui is not None:
                pv_state[ui] = {
                    "po": ppv.tile([P, 512], F32, tag="po", name="po")}

        xt_pending = [True]

        def emit_body():
            # bootstrap: feature chunk 0 projections in [k:0-1024, q:0-1024]
            # order so unit 0's scores start after 2 of the 4 groups; the
            # small weight DMAs are emitted before the big xt DMAs so the
            # first projection matmuls aren't queued behind 4MB of x
            queue_qk(0, k_first=True)
            drain_filler(1)
            if xt_pending[0]:
                # tb-major 128KB pieces: the 8 chunks of the first token
                # block land on 8 DMA engines in parallel, so the first
                # projection matmuls start ~6us in instead of ~23us
                # issue from the (otherwise idle) Pool queue so these 32
                # issues don't serialize behind the weight DMAs on SP
                for tb in range(T // 512):
                    ts = slice(tb * 512, (tb + 1) * 512)
                    for ec in range(EC):
                        nc.gpsimd.dma_start(xt_sb[:, ec, ts], xt_d[ec][:, ts])
                xt_pending[0] = False
            drain_filler(6)
            weave_unit(0)

            queue_qk(1)

            mark('v_proj')
            # v projection (token-major, +bv), interleaved with unit 1's
            # scores+exp and unit 0's PV so ACT stays fed through it
            bvb_v = bvb_sb.rearrange("p (h d) -> p h d", d=D)
            wvh = wv_pool.tile([P, EC, FE], BF, tag="wvh", name="wvh")
            for ec in range(EC):
                nc.sync.dma_start(wvh[:, ec, :], wvt_d[ec])
            for kp in range(KC // 2):
                ps = sc_slot()
                for i in range(2):
                    kc = 2 * kp + i
                    for ec in range(EC):
                        nc.tensor.matmul(
                            ps[:, i, :],
                            lhsT=xt_sb[:, ec, kc * P:(kc + 1) * P],
                            rhs=wvh[:, ec, :],
                            start=(ec == 0), stop=(ec == EC - 1),
                        )
                nc.vector.tensor_tensor(
                    vaug_h[:, 2 * kp: 2 * kp + 2, :, 0:64],
                    ps.rearrange("p a (h d) -> p a h d", d=D),
                    bvb_v[:, None, :, :].to_broadcast((P, 2, HL, D)),
                    ALU.add,
                )
                scores_group(1, kp)
                if kp >= 1:
                    pv_mms(0, 2 * (kp - 1))
                    pv_mms(0, 2 * kp - 1)
                drain_filler(2)
            pv_mms(0, KC - 2)
            pv_mms(0, KC - 1)
            pv_state[1] = {"po": ppv.tile([P, 512], F32, tag="po",
                                          name="po")}
            pv_copy(0)
            norm_pending.append(0)

            mark('attention')
            for fc in range(FCH):
                nc.sync.dma_start(wot_sb[:, fc, :], wot_d[fc])
            nc.sync.dma_start(bob_sb[:], bob_d)

            for ui in range(2, len(UNITS)):
                if ui in (2, 3):          # qk(2) before unit hl=4, qk(3)
                    queue_qk(ui)           # before unit hl=6
                weave_unit(ui)
            weave_unit(len(UNITS) - 1, do_scores=False)
            while norm_pending:
                pv_norm(norm_pending.pop(0))
            drain_filler(100)
            queue_outproj(QB - 1)
            drain_filler(100)

        for _rep in range(REPEAT):
            emit_body()

        mark('tail')
    nc.compile()
    return nc


_NC = None


def _get_nc():
    global _NC
    if _NC is None:
        _NC = build_program()
    return _NC


def _prep_core_inputs(x, Wq, bq, Wk, bk, Wv, bv, Wo, bo):
    """Build the 8 per-core input dicts (host-side sharding)."""
    bf = ml_dtypes.bfloat16
    x = np.asarray(x, dtype=np.float32)
    Wq, Wk, Wv, Wo = (np.asarray(a, np.float32) for a in (Wq, Wk, Wv, Wo))
    bq, bk, bv, bo = (np.asarray(a, np.float32) for a in (bq, bk, bv, bo))
    ones_a = np.ones((P, 64), np.float32)

    halves = []
    for hh in range(2):
        fs = slice(hh * FE, (hh + 1) * FE)
        wqt = np.ascontiguousarray(Wq.T[:, fs]).astype(bf).reshape(EC, P, FE)
        wkt = np.ascontiguousarray(Wk.T[:, fs]).astype(bf).reshape(EC, P, FE)
        wvt = np.ascontiguousarray(Wv.T[:, fs]).astype(bf).reshape(EC, P, FE)
        # wot: own 512 feature ROWS (contraction split), all E cols
        wot = np.ascontiguousarray(Wo.T[fs, :]).astype(bf).reshape(FCH, P, E)
        bq_a = np.ascontiguousarray(bq[fs]).reshape(FCH, P)
        bk_a = np.ascontiguousarray(bk[fs]).reshape(FCH, P)
        bvb = np.ascontiguousarray(
            np.broadcast_to(bv[fs][None, :], (P, FE)))
        # bias folded into the even core's partial; odd adds zeros
        bob = (np.ascontiguousarray(np.broadcast_to(bo[None, :], (P, E)))
               if hh == 0 else np.zeros((P, E), np.float32))
        halves.append(dict(wqt=wqt, wkt=wkt, wvt=wvt, wot=wot, bq=bq_a,
                           bk=bk_a, bvb=bvb, bob=bob))

    in_maps = []
    for c in range(NCORES):
        b, hh = c // 2, c % 2
        hv = halves[hh]
        xt = np.ascontiguousarray(x[b].T).astype(bf).reshape(EC, P, T)
        in_maps.append({
            "xt": xt, "wqt": hv["wqt"], "wkt": hv["wkt"], "wvt": hv["wvt"],
            "wot": hv["wot"], "bq": hv["bq"], "bk": hv["bk"],
            "bvb": hv["bvb"], "bob": hv["bob"], "ones": ones_a,
        })
    return in_maps


def kernel(x, Wq, bq, Wk, bk, Wv, bv, Wo, bo):
    nc = _get_nc()
    in_maps = _prep_core_inputs(x, Wq, bq, Wk, bk, Wv, bv, Wo, bo)
    res = run_bass_kernel_spmd(nc, in_maps, list(range(NCORES)))
    out = np.empty((B, T, E), np.float32)
    for b in range(B):
        p0 = res.results[2 * b]["out"].reshape(T, E)
        p1 = res.results[2 * b + 1]["out"].reshape(T, E)
        out[b] = p0 + p1
    return out
